# revision 25
# baseline (speedup 1.0000x reference)
"""Trainium2 Bass kernel for nn_EvolutionaryFeatureExtractor.

Reference computes, from a one-hot MSA (K=512, L=256, A=21):
  pssm         (L, A)  = log2(((mean + 0.001)/rowsum) * A)
  conservation (L,)    = 1 - entropy/log2(A)
  coevolution  (L, L)  = APC-corrected mutual information

Algebraic restructure (validated against the jax reference):
  joint[i,j,a,b] = C[i,j,a,b]/K + EPS with C the integer pair counts.
  sum_b joint[i,j,a,b] = p_raw[i,a] + A*EPS  (independent of j), so the
  two einsum('ijab,ia->ij') terms collapse to rank-1: MI = S - t_i - t_j
  with S[i,j] = sum_ab J*log2(J) and t[i] = sum_a (p_raw+A*EPS)*log2(p_raw+EPS).
  S is symmetric, so each core only computes a wrap-around band of j.

Device pipeline per core (core c owns i-positions [32c, 32c+32) and the
j-window [32c, 32c+160) mod 256 — band width 160 >= 128+32 covers every
unordered pair from one side or the other):
  counts matmul (fp8, exact for 0/1 one-hots): C^T[jb, ia] = M_win^T M_blk
  ACT:  G = ln(C/512 + 1e-9)
  DVE:  F = (G + SHIFT) * C   (fp16, recentred so fp16 error is tiny;
        the eps*G term is dropped -- ~1e-5 relative, validated)
  PE:   indicator matmul contracts jb partitions -> R[j, ia] = sum_b F
  DVE:  grouped reduce over a -> S_raw^T[j_window, i]
  plus a tiny marginal path for pssm/conservation/t.
Host: gather, S = (S_raw - 512*SHIFT)/(512*ln2), mirror the band across
the diagonal, MI/APC assembly (numpy vector math on gathered results).

The j axis is padded A=21 -> 24 letters so jb rows tile 128 exactly
(pad rows have C=0 and F=0 so they are self-masking).
"""

import numpy as np
import ml_dtypes

import concourse.bass as bass
import concourse.mybir as mybir
import concourse.tile as tile
from concourse import bacc, bass_utils

F32 = mybir.dt.float32
F16 = mybir.dt.float16
FP8 = mybir.dt.float8e4  # e4m3: 0.0/1.0 exact
NP_FP8 = ml_dtypes.float8_e4m3

K, L, A = 512, 256, 21
AP24 = 24                  # padded alphabet for the jb (partition) axis
N_CORES = 8
IB = L // N_CORES          # 32 i-positions per core
NW = IB * A                # 672 rhs columns per core
JW = 160                   # j-window positions per core (wrap-around band)
MP = JW * AP24             # 3840 lhsT columns (jb, padded)
NPT = MP // 128            # 30 jb partition tiles
KT = K // 128              # 4 contraction tiles
KT2 = 2                    # DoubleRow contraction tiles (256 each)
NCH = 5                    # lhsT DMA chunks
CPT = NPT // NCH           # 6 ptiles per chunk
EPS = 1e-9
SHIFT = 6.0                # F = (ln J + SHIFT) * C recentring
LN2 = float(np.log(2.0))

_CACHE = {}


def _build():
    nc = bacc.Bacc("TRN2", target_bir_lowering=False, debug=False,
                   num_devices=N_CORES)

    # host-pretiled layouts: partition dim first, big contiguous runs.
    # k is pair-interleaved for DoubleRow: slot (t, p, j) holds original
    # k = 256t + 2p + j on BOTH operands (contraction is permutation-invariant)
    lhst_d = nc.dram_tensor("lhst", [128, NCH, KT2, 2, CPT * 128], FP8,
                            kind="ExternalInput").ap()
    rhs_d = nc.dram_tensor("rhs", [128, KT2, 2, NW], FP8, kind="ExternalInput").ap()
    sout_d = nc.dram_tensor("sout", [JW, IB], F32, kind="ExternalOutput").ap()
    pssm_d = nc.dram_tensor("pssm_raw", [1, NW], F32, kind="ExternalOutput").ap()
    traw_d = nc.dram_tensor("traw", [1, IB], F32, kind="ExternalOutput").ap()
    eraw_d = nc.dram_tensor("eraw", [1, IB], F32, kind="ExternalOutput").ap()

    with tile.TileContext(nc) as tc:
        with (
            tc.tile_pool(name="inp", bufs=1) as inp,
            tc.tile_pool(name="cpool", bufs=3, space="PSUM") as cpool,
            tc.tile_pool(name="rpool", bufs=1, space="PSUM") as rpool,
            tc.tile_pool(name="gpool", bufs=2) as gpool,
            tc.tile_pool(name="fpool", bufs=6) as fpool,
            tc.tile_pool(name="opool", bufs=1) as opool,
        ):
            # input DMAs spread across engine DGE queues so they overlap;
            # first-needed pieces (rbuf, kchunk0) split per k-tile so the
            # earliest matmuls can start as soon as their slice lands
            bias1 = inp.tile([128, 1], F32)
            nc.vector.memset(bias1[:], EPS)
            biasp = inp.tile([128, 1], F32)
            nc.vector.memset(biasp[:], 0.001 * A / (1.0 + A * 0.001))
            ones8 = inp.tile([128, 32], FP8)
            nc.vector.memset(ones8[:], 1.0)
            scratch = inp.tile([128, 256], FP8)
            nc.vector.memset(scratch[:], 0.0)

            # indicator generated on-device (saves 786KB of DMA). The 24
            # per-ptile patterns repeat every 3 ptiles shifted by 16 output
            # rows, so only 3 distinct patterns are built, padded to 240
            # cols; M1 slices [112-16q : 240-16q].
            # P[p, s, x] = 1 iff 0 <= 128s + p - 24(x-112) <= 23
            ibuf = inp.tile([128, 3, 240], F16)
            nc.vector.memset(ibuf[:], 1.0)
            nc.gpsimd.affine_select(ibuf[:], ibuf[:],
                                    pattern=[[128, 3], [-AP24, 240]],
                                    compare_op=mybir.AluOpType.is_ge,
                                    fill=0.0, base=AP24 * 112,
                                    channel_multiplier=1)
            nc.gpsimd.affine_select(ibuf[:], ibuf[:],
                                    pattern=[[-128, 3], [AP24, 240]],
                                    compare_op=mybir.AluOpType.is_ge,
                                    fill=0.0, base=(AP24 - 1) - AP24 * 112,
                                    channel_multiplier=-1)

            rbuf = inp.tile([128, KT2, 2, NW], FP8)
            nc.sync.dma_start(rbuf[:], rhs_d[:])
            rbufs = [rbuf[:, t] for t in range(KT2)]
            kc0 = inp.tile([128, KT2, 2, CPT * 128], FP8, name="kchunk0")
            nc.scalar.dma_start(kc0[:], lhst_d[:, 0, :, :, :])

            kc1 = inp.tile([128, KT2, 2, CPT * 128], FP8, name="kchunk1")
            nc.gpsimd.dma_start(kc1[:], lhst_d[:, 1, :, :, :])
            kc2 = inp.tile([128, KT2, 2, CPT * 128], FP8, name="kchunk2")
            nc.scalar.dma_start(kc2[:], lhst_d[:, 2, :, :, :])
            kc3 = inp.tile([128, KT2, 2, CPT * 128], FP8, name="kchunk3")
            nc.sync.dma_start(kc3[:], lhst_d[:, 3, :, :, :])
            kc4 = inp.tile([128, KT2, 2, CPT * 128], FP8, name="kchunk4")
            nc.sync.dma_start(kc4[:], lhst_d[:, 4, :, :, :])
            kchunks = [kc0, kc1, kc2, kc3, kc4]

            # r1a accumulates j 0..127 (ptiles 0..23) for the whole loop;
            # r1b (j 128..159, ptiles 24..29) is allocated lazily from the
            # C pool so three C slots fit in PSUM during iterations 0..23
            r1a = rpool.tile([128, 1024], F32, tag="r1a")
            r1 = {}

            # HAM warmup: garbage-data matmuls with no dependencies so the
            # PE clock ramps to 8/8 while input DMAs are in flight; they
            # write the r1a bank, which the first real M1 (start=True)
            # clears before any reader. Extra warmups are sprinkled through
            # the DMA-bound head so arrival gaps don't re-throttle the clock.
            def warm_mm(n):
                for _ in range(n):
                    nc.tensor.matmul(r1a[:, 0:256], scratch[:, 0:128], scratch[:],
                                     start=True, stop=True)

            warm_mm(12)

            # marginal path next: needs only rbuf, keeps PE busy while the
            # big lhsT chunks are still in flight (DoubleRow with a stride-16
            # ones [128, 2, 1] weight AP)
            cm = cpool.tile([1, 1024], F32, tag="c", name="cm")
            onesdr = ones8[:].rearrange("p (a b) -> p a b", b=16)[:, :, 0:1]
            for t in range(KT2):
                nc.tensor.matmul(cm[:, 0:512], onesdr, rbufs[t][:, :, 0:512],
                                 start=(t == 0), stop=(t == KT2 - 1),
                                 perf_mode=mybir.MatmulPerfMode.DoubleRow)
                nc.tensor.matmul(cm[:, 512:NW], onesdr, rbufs[t][:, :, 512:NW],
                                 start=(t == 0), stop=(t == KT2 - 1),
                                 perf_mode=mybir.MatmulPerfMode.DoubleRow)
            warm_mm(6)
            lnfe = gpool.tile([1, NW], F32, tag="g", name="lnfe")
            nc.scalar.activation(lnfe[:], cm[:, 0:NW],
                                 mybir.ActivationFunctionType.Ln,
                                 scale=1.0 / K, bias=bias1[0:1])
            pssm_t = fpool.tile([1, NW], F32, tag="f", name="pssm_t")
            nc.scalar.activation(pssm_t[:], cm[:, 0:NW],
                                 mybir.ActivationFunctionType.Ln,
                                 scale=A / (K * (1.0 + A * 0.001)),
                                 bias=biasp[0:1])
            nc.sync.dma_start(pssm_d[:, :], pssm_t[:])
            tv = fpool.tile([1, NW], F32, tag="f", name="tv")
            nc.vector.scalar_tensor_tensor(
                tv[:], cm[:, 0:NW], float(K * A * EPS), lnfe[:],
                op0=mybir.AluOpType.add, op1=mybir.AluOpType.mult)
            ev = gpool.tile([1, NW], F32, tag="g", name="ev")
            nc.vector.scalar_tensor_tensor(
                ev[:], cm[:, 0:NW], float(K * EPS), lnfe[:],
                op0=mybir.AluOpType.add, op1=mybir.AluOpType.mult)
            tr = opool.tile([1, IB], F32)
            nc.vector.reduce_sum(tr[:], tv[:].rearrange("p (i a) -> p i a", a=A),
                                 axis=mybir.AxisListType.X)
            er = opool.tile([1, IB], F32)
            nc.vector.reduce_sum(er[:], ev[:].rearrange("p (i a) -> p i a", a=A),
                                 axis=mybir.AxisListType.X)
            nc.sync.dma_start(traw_d[:, :], tr[:])
            nc.sync.dma_start(eraw_d[:, :], er[:])

            # main loop; the indicator matmul for ptile r is emitted DELAY
            # iterations later so it never head-of-line-blocks the PE queue
            # waiting on the DVE to produce F
            DELAY = 2
            fs = [None] * NPT

            def emit_m1(r):
                if r < 24:
                    rt = r1a
                else:
                    if "b" not in r1:
                        r1["b"] = cpool.tile([128, 1024], F32, tag="c", name="r1b")
                    rt = r1["b"]
                s = (r % 24) % 3
                q = (r % 24) // 3
                ind_ap = ibuf[:, s, 112 - 16 * q: 240 - 16 * q]
                nc.tensor.matmul(rt[:, 0:512], ind_ap, fs[r][:, 0:512],
                                 start=(r % 24 == 0),
                                 stop=(r % 24 == 23 or r == NPT - 1))
                nc.tensor.matmul(rt[:, 512:NW], ind_ap, fs[r][:, 512:NW],
                                 start=(r % 24 == 0),
                                 stop=(r % 24 == 23 or r == NPT - 1))

            for r in range(NPT):
                c_idx = r // CPT
                co = (r % CPT) * 128
                ctile = cpool.tile([128, 1024], F32, tag="c", name=f"c{r}")
                for t in range(KT2):
                    lw = kchunks[c_idx][:, t, :, co:co + 128]
                    nc.tensor.matmul(ctile[:, 0:512], lw, rbufs[t][:, :, 0:512],
                                     start=(t == 0), stop=(t == KT2 - 1),
                                     perf_mode=mybir.MatmulPerfMode.DoubleRow)
                    nc.tensor.matmul(ctile[:, 512:NW], lw, rbufs[t][:, :, 512:NW],
                                     start=(t == 0), stop=(t == KT2 - 1),
                                     perf_mode=mybir.MatmulPerfMode.DoubleRow)
                if 0 < r < DELAY:
                    warm_mm(3)
                if r >= DELAY:
                    emit_m1(r - DELAY)
                g = gpool.tile([128, NW], F32, tag="g", name=f"g{r}")
                nc.scalar.activation(g[:], ctile[:, 0:NW],
                                     mybir.ActivationFunctionType.Ln,
                                     scale=1.0 / K, bias=bias1[:])
                f = fpool.tile([128, NW], F16, tag="f", name=f"f{r}")
                fs[r] = f
                nc.vector.scalar_tensor_tensor(
                    f[:], g[:], SHIFT, ctile[:, 0:NW],
                    op0=mybir.AluOpType.add, op1=mybir.AluOpType.mult)
            for r in range(NPT - DELAY, NPT):
                emit_m1(r)

            sa = opool.tile([128, IB], F32)
            nc.vector.reduce_sum(sa[:], r1a[:, 0:NW].rearrange("p (i a) -> p i a", a=A),
                                 axis=mybir.AxisListType.X)
            sb = opool.tile([32, IB], F32)
            nc.vector.reduce_sum(sb[:], r1["b"][0:32, 0:NW].rearrange("p (i a) -> p i a", a=A),
                                 axis=mybir.AxisListType.X)
            nc.sync.dma_start(sout_d[0:128, :], sa[:])
            nc.sync.dma_start(sout_d[128:JW, :], sb[:])

    nc.compile()
    return nc


def _indicator():
    # ind[p, r, j] = 1 iff (128*r + p) // 24 == j (mod 128 within group)
    ind = np.zeros((128, 24, 128), np.float16)
    r = np.arange(24)[None, :]
    p = np.arange(128)[:, None]
    j = (128 * r + p) // AP24
    ind[p, r, j] = 1.0
    return ind


def prep_inputs(msa: np.ndarray):
    msa = np.asarray(msa, dtype=np.float32)
    m21 = msa.reshape(K, L * A).astype(NP_FP8)
    m24 = np.zeros((K, L, AP24), NP_FP8)
    m24[:, :, :A] = msa

    in_maps = []
    for c in range(N_CORES):
        jpos = (32 * c + np.arange(JW)) % L
        win = m24[:, jpos, :].reshape(K, MP)          # [512, 3840]
        # DoubleRow pair-interleave: slot (t, p, j) = original k 256t + 2p + j
        lh = np.ascontiguousarray(
            win.reshape(KT2, 128, 2, NCH, CPT * 128).transpose(1, 3, 0, 2, 4))
        rh = np.ascontiguousarray(
            m21[:, c * NW:(c + 1) * NW].reshape(KT2, 128, 2, NW).transpose(1, 0, 2, 3))
        in_maps.append({"lhst": lh, "rhs": rh})
    return in_maps


def run(msa: np.ndarray, trace: bool = False):
    """Shard, run the SPMD kernel on 8 cores, return BassKernelResults."""
    if "nc" not in _CACHE:
        _CACHE["nc"] = _build()
    return bass_utils.run_bass_kernel_spmd(
        _CACHE["nc"], prep_inputs(msa), core_ids=list(range(N_CORES)), trace=trace)


def assemble(res) -> tuple[np.ndarray, np.ndarray, np.ndarray]:
    inv = 1.0 / (K * LN2)
    # core c: sout[jw, i_local] -> S[32c + i_local, (32c + jw) % L]
    s = np.zeros((L, L), np.float32)
    covered = np.zeros((L, L), bool)
    jw = np.arange(JW)
    for c in range(N_CORES):
        jpos = (32 * c + jw) % L
        blk = res.results[c]["sout"]                  # [JW, IB]
        s[32 * c:32 * c + IB, jpos] = blk.T
        covered[32 * c:32 * c + IB, jpos] = True
    s = np.where(covered, s, s.T)
    s = (s - np.float32(K * SHIFT)) * np.float32(inv)

    t = np.concatenate([res.results[c]["traw"][0] for c in range(N_CORES)]) * np.float32(inv)
    e = np.concatenate([res.results[c]["eraw"][0] for c in range(N_CORES)])
    pssm = np.concatenate([res.results[c]["pssm_raw"][0] for c in range(N_CORES)])

    mi = s - t[:, None] - t[None, :]
    np.fill_diagonal(mi, 0.0)
    rm = mi.mean(axis=1)
    cm = mi.mean(axis=0)
    tm = mi.mean()
    mi = mi - np.outer(rm, cm) / (tm + EPS)

    pssm = (pssm / np.float32(LN2)).reshape(L, A).astype(np.float32)
    conservation = (1.0 + e * inv / np.log2(A)).astype(np.float32)
    return pssm, conservation, mi.astype(np.float32)


def kernel(msa: np.ndarray):
    res = run(np.asarray(msa, dtype=np.float32), trace=False)
    return assemble(res)


# revision 26
# speedup vs baseline: 1.0587x; 1.0587x over previous
"""Trainium2 Bass kernel for nn_EvolutionaryFeatureExtractor.

Reference computes, from a one-hot MSA (K=512, L=256, A=21):
  pssm         (L, A)  = log2(((mean + 0.001)/rowsum) * A)
  conservation (L,)    = 1 - entropy/log2(A)
  coevolution  (L, L)  = APC-corrected mutual information

Algebraic restructure (validated against the jax reference):
  joint[i,j,a,b] = C[i,j,a,b]/K + EPS with C the integer pair counts.
  sum_b joint[i,j,a,b] = p_raw[i,a] + A*EPS  (independent of j), so the
  two einsum('ijab,ia->ij') terms collapse to rank-1: MI = S - t_i - t_j
  with S[i,j] = sum_ab J*log2(J) and t[i] = sum_a (p_raw+A*EPS)*log2(p_raw+EPS).
  S is symmetric, so each core only computes a wrap-around band of j.

Device pipeline per core (core c owns i-positions [32c, 32c+32) and the
j-window [32c, 32c+160) mod 256 — band width 160 >= 128+32 covers every
unordered pair from one side or the other):
  counts matmul (fp8, exact for 0/1 one-hots): C^T[jb, ia] = M_win^T M_blk
  ACT:  G = ln(C/512 + 1e-9)
  DVE:  F = (G + SHIFT) * C   (fp16, recentred so fp16 error is tiny;
        the eps*G term is dropped -- ~1e-5 relative, validated)
  PE:   indicator matmul contracts jb partitions -> R[j, ia] = sum_b F
  DVE:  grouped reduce over a -> S_raw^T[j_window, i]
  plus a tiny marginal path for pssm/conservation/t.
Host: gather, S = (S_raw - 512*SHIFT)/(512*ln2), mirror the band across
the diagonal, MI/APC assembly (numpy vector math on gathered results).

The j axis is padded A=21 -> 24 letters so jb rows tile 128 exactly
(pad rows have C=0 and F=0 so they are self-masking).
"""

import numpy as np
import ml_dtypes

import concourse.bass as bass
import concourse.mybir as mybir
import concourse.tile as tile
from concourse import bacc, bass_utils

F32 = mybir.dt.float32
F16 = mybir.dt.float16
FP8 = mybir.dt.float8e4  # e4m3: 0.0/1.0 exact
NP_FP8 = ml_dtypes.float8_e4m3

K, L, A = 512, 256, 21
AP24 = 24                  # padded alphabet for the jb (partition) axis
N_CORES = 8
IB = L // N_CORES          # 32 i-positions per core
NW = IB * A                # 672 rhs columns per core
JW = 160                   # j-window positions per core (wrap-around band)
MP = JW * AP24             # 3840 lhsT columns (jb, padded)
NPT = MP // 128            # 30 jb partition tiles
KT = K // 128              # 4 contraction tiles
KT2 = 2                    # DoubleRow contraction tiles (256 each)
NCH = 5                    # lhsT DMA chunks
CPT = NPT // NCH           # 6 ptiles per chunk
EPS = 1e-9
SHIFT = 6.0                # F = (ln J + SHIFT) * C recentring
LN2 = float(np.log(2.0))

_CACHE = {}


def _build():
    nc = bacc.Bacc("TRN2", target_bir_lowering=False, debug=False,
                   num_devices=N_CORES)

    # host-pretiled layouts: partition dim first, big contiguous runs.
    # k is pair-interleaved for DoubleRow: slot (t, p, j) holds original
    # k = 256t + 2p + j on BOTH operands (contraction is permutation-invariant)
    lhst_d = nc.dram_tensor("lhst", [128, NCH, KT2, 2, CPT * 128], FP8,
                            kind="ExternalInput").ap()
    rhs_d = nc.dram_tensor("rhs", [128, KT2, 2, NW], FP8, kind="ExternalInput").ap()
    sout_d = nc.dram_tensor("sout", [JW, IB], F32, kind="ExternalOutput").ap()
    pssm_d = nc.dram_tensor("pssm_raw", [1, NW], F32, kind="ExternalOutput").ap()
    traw_d = nc.dram_tensor("traw", [1, IB], F32, kind="ExternalOutput").ap()
    eraw_d = nc.dram_tensor("eraw", [1, IB], F32, kind="ExternalOutput").ap()

    with tile.TileContext(nc) as tc:
        with (
            tc.tile_pool(name="inp", bufs=1) as inp,
            tc.tile_pool(name="cpool", bufs=3, space="PSUM") as cpool,
            tc.tile_pool(name="rpool", bufs=1, space="PSUM") as rpool,
            tc.tile_pool(name="gpool", bufs=2) as gpool,
            tc.tile_pool(name="fpool", bufs=6) as fpool,
            tc.tile_pool(name="opool", bufs=1) as opool,
        ):
            # input DMAs spread across engine DGE queues so they overlap;
            # first-needed pieces (rbuf, kchunk0) split per k-tile so the
            # earliest matmuls can start as soon as their slice lands
            bias1 = inp.tile([128, 1], F32)
            nc.vector.memset(bias1[:], EPS)
            biasp = inp.tile([128, 1], F32)
            nc.vector.memset(biasp[:], 0.001 * A / (1.0 + A * 0.001))
            ones8 = inp.tile([128, 32], FP8)
            nc.vector.memset(ones8[:], 1.0)
            scratch = inp.tile([128, 256], FP8)
            nc.vector.memset(scratch[:], 0.0)

            # indicator generated on-device (saves 786KB of DMA). The 24
            # per-ptile patterns repeat every 3 ptiles shifted by 16 output
            # rows, so only 3 distinct patterns are built, padded to 240
            # cols; M1 slices [112-16q : 240-16q].
            # P[p, s, x] = 1 iff 0 <= 128s + p - 24(x-112) <= 23
            ibuf = inp.tile([128, 3, 240], F16)
            nc.vector.memset(ibuf[:], 1.0)
            nc.gpsimd.affine_select(ibuf[:], ibuf[:],
                                    pattern=[[128, 3], [-AP24, 240]],
                                    compare_op=mybir.AluOpType.is_ge,
                                    fill=0.0, base=AP24 * 112,
                                    channel_multiplier=1)
            nc.gpsimd.affine_select(ibuf[:], ibuf[:],
                                    pattern=[[-128, 3], [AP24, 240]],
                                    compare_op=mybir.AluOpType.is_ge,
                                    fill=0.0, base=(AP24 - 1) - AP24 * 112,
                                    channel_multiplier=-1)

            rbuf = inp.tile([128, KT2, 2, NW], FP8)
            nc.sync.dma_start(rbuf[:], rhs_d[:])
            rbufs = [rbuf[:, t] for t in range(KT2)]
            kc0 = inp.tile([128, KT2, 2, CPT * 128], FP8, name="kchunk0")
            nc.scalar.dma_start(kc0[:], lhst_d[:, 0, :, :, :])

            kc1 = inp.tile([128, KT2, 2, CPT * 128], FP8, name="kchunk1")
            nc.gpsimd.dma_start(kc1[:], lhst_d[:, 1, :, :, :])
            kc2 = inp.tile([128, KT2, 2, CPT * 128], FP8, name="kchunk2")
            nc.scalar.dma_start(kc2[:], lhst_d[:, 2, :, :, :])
            kc3 = inp.tile([128, KT2, 2, CPT * 128], FP8, name="kchunk3")
            nc.sync.dma_start(kc3[:], lhst_d[:, 3, :, :, :])
            kc4 = inp.tile([128, KT2, 2, CPT * 128], FP8, name="kchunk4")
            nc.sync.dma_start(kc4[:], lhst_d[:, 4, :, :, :])
            kchunks = [kc0, kc1, kc2, kc3, kc4]

            # r1a accumulates j 0..127 (ptiles 0..23) for the whole loop;
            # r1b (j 128..159, ptiles 24..29) is allocated lazily from the
            # C pool so three C slots fit in PSUM during iterations 0..23
            r1a = rpool.tile([128, 1024], F32, tag="r1a")
            r1 = {}

            # HAM warmup: garbage-data matmuls with no dependencies so the
            # PE clock ramps to 8/8 while input DMAs are in flight; they
            # write the r1a bank, which the first real M1 (start=True)
            # clears before any reader. Extra warmups are sprinkled through
            # the DMA-bound head so arrival gaps don't re-throttle the clock.
            def warm_mm(n):
                for _ in range(n):
                    nc.tensor.matmul(r1a[:, 0:256], scratch[:, 0:128], scratch[:],
                                     start=True, stop=True)

            warm_mm(12)

            # marginal path next: needs only rbuf, keeps PE busy while the
            # big lhsT chunks are still in flight (DoubleRow with a stride-16
            # ones [128, 2, 1] weight AP)
            cm = cpool.tile([1, 1024], F32, tag="c", name="cm")
            onesdr = ones8[:].rearrange("p (a b) -> p a b", b=16)[:, :, 0:1]
            for t in range(KT2):
                nc.tensor.matmul(cm[:, 0:512], onesdr, rbufs[t][:, :, 0:512],
                                 start=(t == 0), stop=(t == KT2 - 1),
                                 perf_mode=mybir.MatmulPerfMode.DoubleRow)
                nc.tensor.matmul(cm[:, 512:NW], onesdr, rbufs[t][:, :, 512:NW],
                                 start=(t == 0), stop=(t == KT2 - 1),
                                 perf_mode=mybir.MatmulPerfMode.DoubleRow)
            warm_mm(6)
            lnfe = gpool.tile([1, NW], F32, tag="g", name="lnfe")
            nc.scalar.activation(lnfe[:], cm[:, 0:NW],
                                 mybir.ActivationFunctionType.Ln,
                                 scale=1.0 / K, bias=bias1[0:1])
            pssm_t = fpool.tile([1, NW], F32, tag="f", name="pssm_t")
            nc.scalar.activation(pssm_t[:], cm[:, 0:NW],
                                 mybir.ActivationFunctionType.Ln,
                                 scale=A / (K * (1.0 + A * 0.001)),
                                 bias=biasp[0:1])
            nc.sync.dma_start(pssm_d[:, :], pssm_t[:])
            tv = fpool.tile([1, NW], F32, tag="f", name="tv")
            nc.vector.scalar_tensor_tensor(
                tv[:], cm[:, 0:NW], float(K * A * EPS), lnfe[:],
                op0=mybir.AluOpType.add, op1=mybir.AluOpType.mult)
            ev = gpool.tile([1, NW], F32, tag="g", name="ev")
            nc.vector.scalar_tensor_tensor(
                ev[:], cm[:, 0:NW], float(K * EPS), lnfe[:],
                op0=mybir.AluOpType.add, op1=mybir.AluOpType.mult)
            tr = opool.tile([1, IB], F32)
            nc.vector.reduce_sum(tr[:], tv[:].rearrange("p (i a) -> p i a", a=A),
                                 axis=mybir.AxisListType.X)
            er = opool.tile([1, IB], F32)
            nc.vector.reduce_sum(er[:], ev[:].rearrange("p (i a) -> p i a", a=A),
                                 axis=mybir.AxisListType.X)
            nc.sync.dma_start(traw_d[:, :], tr[:])
            nc.sync.dma_start(eraw_d[:, :], er[:])

            # main loop; the indicator matmul for ptile r is emitted DELAY
            # iterations later so it never head-of-line-blocks the PE queue
            # waiting on the DVE to produce F
            DELAY = 3
            fs = [None] * NPT

            def emit_m1(r):
                if r < 24:
                    rt = r1a
                else:
                    if "b" not in r1:
                        r1["b"] = cpool.tile([128, 1024], F32, tag="c", name="r1b")
                    rt = r1["b"]
                s = (r % 24) % 3
                q = (r % 24) // 3
                ind_ap = ibuf[:, s, 112 - 16 * q: 240 - 16 * q]
                nc.tensor.matmul(rt[:, 0:512], ind_ap, fs[r][:, 0:512],
                                 start=(r % 24 == 0),
                                 stop=(r % 24 == 23 or r == NPT - 1))
                nc.tensor.matmul(rt[:, 512:NW], ind_ap, fs[r][:, 512:NW],
                                 start=(r % 24 == 0),
                                 stop=(r % 24 == 23 or r == NPT - 1))

            for r in range(NPT):
                c_idx = r // CPT
                co = (r % CPT) * 128
                ctile = cpool.tile([128, 1024], F32, tag="c", name=f"c{r}")
                for t in range(KT2):
                    lw = kchunks[c_idx][:, t, :, co:co + 128]
                    nc.tensor.matmul(ctile[:, 0:512], lw, rbufs[t][:, :, 0:512],
                                     start=(t == 0), stop=(t == KT2 - 1),
                                     perf_mode=mybir.MatmulPerfMode.DoubleRow)
                    nc.tensor.matmul(ctile[:, 512:NW], lw, rbufs[t][:, :, 512:NW],
                                     start=(t == 0), stop=(t == KT2 - 1),
                                     perf_mode=mybir.MatmulPerfMode.DoubleRow)
                if 0 < r < DELAY:
                    warm_mm(3)
                if r >= DELAY:
                    emit_m1(r - DELAY)
                g = gpool.tile([128, NW], F32, tag="g", name=f"g{r}")
                nc.scalar.activation(g[:], ctile[:, 0:NW],
                                     mybir.ActivationFunctionType.Ln,
                                     scale=1.0 / K, bias=bias1[:])
                f = fpool.tile([128, NW], F16, tag="f", name=f"f{r}")
                fs[r] = f
                nc.vector.scalar_tensor_tensor(
                    f[:], g[:], SHIFT, ctile[:, 0:NW],
                    op0=mybir.AluOpType.add, op1=mybir.AluOpType.mult)
            for r in range(NPT - DELAY, NPT):
                emit_m1(r)

            sa = opool.tile([128, IB], F32)
            nc.vector.reduce_sum(sa[:], r1a[:, 0:NW].rearrange("p (i a) -> p i a", a=A),
                                 axis=mybir.AxisListType.X)
            sb = opool.tile([32, IB], F32)
            nc.vector.reduce_sum(sb[:], r1["b"][0:32, 0:NW].rearrange("p (i a) -> p i a", a=A),
                                 axis=mybir.AxisListType.X)
            nc.sync.dma_start(sout_d[0:128, :], sa[:])
            nc.sync.dma_start(sout_d[128:JW, :], sb[:])

    nc.compile()
    return nc


def _indicator():
    # ind[p, r, j] = 1 iff (128*r + p) // 24 == j (mod 128 within group)
    ind = np.zeros((128, 24, 128), np.float16)
    r = np.arange(24)[None, :]
    p = np.arange(128)[:, None]
    j = (128 * r + p) // AP24
    ind[p, r, j] = 1.0
    return ind


def prep_inputs(msa: np.ndarray):
    msa = np.asarray(msa, dtype=np.float32)
    m21 = msa.reshape(K, L * A).astype(NP_FP8)
    m24 = np.zeros((K, L, AP24), NP_FP8)
    m24[:, :, :A] = msa

    in_maps = []
    for c in range(N_CORES):
        jpos = (32 * c + np.arange(JW)) % L
        win = m24[:, jpos, :].reshape(K, MP)          # [512, 3840]
        # DoubleRow pair-interleave: slot (t, p, j) = original k 256t + 2p + j
        lh = np.ascontiguousarray(
            win.reshape(KT2, 128, 2, NCH, CPT * 128).transpose(1, 3, 0, 2, 4))
        rh = np.ascontiguousarray(
            m21[:, c * NW:(c + 1) * NW].reshape(KT2, 128, 2, NW).transpose(1, 0, 2, 3))
        in_maps.append({"lhst": lh, "rhs": rh})
    return in_maps


def run(msa: np.ndarray, trace: bool = False):
    """Shard, run the SPMD kernel on 8 cores, return BassKernelResults."""
    if "nc" not in _CACHE:
        _CACHE["nc"] = _build()
    return bass_utils.run_bass_kernel_spmd(
        _CACHE["nc"], prep_inputs(msa), core_ids=list(range(N_CORES)), trace=trace)


def assemble(res) -> tuple[np.ndarray, np.ndarray, np.ndarray]:
    inv = 1.0 / (K * LN2)
    # core c: sout[jw, i_local] -> S[32c + i_local, (32c + jw) % L]
    s = np.zeros((L, L), np.float32)
    covered = np.zeros((L, L), bool)
    jw = np.arange(JW)
    for c in range(N_CORES):
        jpos = (32 * c + jw) % L
        blk = res.results[c]["sout"]                  # [JW, IB]
        s[32 * c:32 * c + IB, jpos] = blk.T
        covered[32 * c:32 * c + IB, jpos] = True
    s = np.where(covered, s, s.T)
    s = (s - np.float32(K * SHIFT)) * np.float32(inv)

    t = np.concatenate([res.results[c]["traw"][0] for c in range(N_CORES)]) * np.float32(inv)
    e = np.concatenate([res.results[c]["eraw"][0] for c in range(N_CORES)])
    pssm = np.concatenate([res.results[c]["pssm_raw"][0] for c in range(N_CORES)])

    mi = s - t[:, None] - t[None, :]
    np.fill_diagonal(mi, 0.0)
    rm = mi.mean(axis=1)
    cm = mi.mean(axis=0)
    tm = mi.mean()
    mi = mi - np.outer(rm, cm) / (tm + EPS)

    pssm = (pssm / np.float32(LN2)).reshape(L, A).astype(np.float32)
    conservation = (1.0 + e * inv / np.log2(A)).astype(np.float32)
    return pssm, conservation, mi.astype(np.float32)


def kernel(msa: np.ndarray):
    res = run(np.asarray(msa, dtype=np.float32), trace=False)
    return assemble(res)
